# revision 34
# baseline (speedup 1.0000x reference)
"""BDS-vote (bidirectional NNF patch voting) Trainium2 kernel.

Strategy (v2: whole-patch items over a row-triple layout)
---------------------------------------------------------
A center pair (t, q, w) contributes guide[t+d] += w*ref[q+d] for the 9
offsets d of a 3x3 patch.  Store ref as row triples:

    REF3[p] = [ref[p-W], ref[p], ref[p+W]]          (3C channels, 768B/px)

(zero-padded at the image's top/bottom rows).  Then the WHOLE 3x3 patch
contribution of one center is a single contiguous 2304B item:

    GUIDE3[ty, tx-1:tx+2] += w * REF3[qy, qx-1:qx+2]

where GUIDE3 has the same triple layout and folds back to guide via a
vertical stencil:  guide[y] = GUIDE3[y-1][slot2] + GUIDE3[y][slot1] +
GUIDE3[y+1][slot0].  Per-pixel validity is exact: x-border centers are
diverted to a pixel-granular stream, y-validity comes from REF3's zero
edge slots plus the fold's row range.

Items map 1:1 onto GPSIMD SWDGE ops (dma_gather / dma_scatter_add) with
2304B elements — 3x fewer items than per-(dy-row) segments.

Sharding: output-row slabs.  Core k owns target rows [64k, 64k+64); its
GUIDE3 slab carries one halo row on each side, and items whose center
row is a slab boundary row are duplicated to the neighbour core (each
core's fold only reads the slots it owns, so nothing is double counted).
The weight plane is pure int-map arithmetic -> host bincount; the final
divide is a fused on-device multiply by 1/weight.

SPMD: one program for all 8 cores.  Chunk sizes are the cross-core max
per bucket; cores pad with dummy items that gather row 0 and scatter
into trash rows outside the real slab.

Timing: the axon-proxied dispatch has ~80ms of network round-trip
latency that fully overlaps across back-to-back executions, while the
device executions serialize — so HW exec time is measured as the
marginal time of N pipelined runs: (T(N) - T(1)) / (N-1).
"""

import os
import sys

for p in ("/opt/trn_rl_repo",):
    if p not in sys.path:
        sys.path.insert(0, p)

import numpy as np

# ---------------------------------------------------------------- params
C = 64
C3 = 3 * C                      # REF3/GUIDE3 channels
H = 512
W = 512
HW = H * W
NCORES = 8
SLABROWS = H // NCORES          # 64 target rows per core
SLABPX = SLABROWS * W           # 32768
NWIN = 8
WINPX = HW // NWIN              # 32768 pixel-rows per gather window
WINROWS = H // NWIN             # for the px stream (refws windows)
PATCH = 3
COMPLETENESS = 2.0
WS = 1.0 / (HW)
WR = COMPLETENESS / (HW)

# GUIDE3 slab: 66 image rows (64 owned + 1 halo row each side).
S3ROWS = SLABROWS + 2
SLAB3PX = S3ROWS * W            # 33792 pixel-rows
TRASH3 = 64
HPX3 = SLAB3PX // 2             # 16896; int16 windowing of the scatter side
A3_ROWS = TRASH3 + SLAB3PX + TRASH3 + 16
SVIEW3 = HPX3 + TRASH3 + 8

# px stream accumulator (pixel-granular, C channels) — baseline layout.
TRASHROWS = 64
HPX = SLABPX // 2               # 16384
REAL0 = TRASHROWS
A_ROWS = TRASHROWS + SLABPX + TRASHROWS + 16
SVIEW_ROWS = HPX + TRASHROWS + 8

REFM = 512                      # zero margin rows around refws (y-edge slots)
REF_ROWS = REFM + HW + REFM + 16
REF3_ROWS = HW + 16

# SWDGE instructions above ~512 items overflow the descriptor ring on HW
# (NRT_EXEC_UNIT_UNRECOVERABLE) — keep chunks at 512.
PAT_CH = 512
PX_CH = 512

F32 = np.float32
I16 = np.int16


# ---------------------------------------------------------------- host prep
def _wrap16(idx: np.ndarray) -> np.ndarray:
    """[N] int -> [128, N/16] int16 SWDGE index layout (idx j at [j%16, j//16],
    replicated across the 8 GPSIMD q7 cores).  -1 entries are padding the HW
    skips (trailing negative indices are ignored by dma_gather/scatter_add)."""
    n = idx.shape[0]
    assert n % 16 == 0
    assert idx.min() >= -1 and idx.max() < 32768, (idx.min(), idx.max())
    blk = idx.reshape(n // 16, 16).T.astype(I16)     # [16, n/16]
    return np.tile(blk, (8, 1))                      # [128, n/16]


def _layerize(g, s, width):
    """Split a bucket into layers with pairwise-disjoint scatter windows.

    HW dma_scatter_add loses adds when two descriptors of one instruction
    write overlapping ranges, so every instruction must be conflict-free.
    Layer id = dup_rank * width + (s mod width): same layer => same phase
    (starts differ by a multiple of width => windows disjoint unless
    equal) and same dup rank (equal starts get distinct ranks)."""
    if len(s) == 0:
        return []
    o = np.argsort(s, kind="stable")
    s_o = s[o]
    idx = np.arange(len(s))
    new_run = np.ones(len(s), bool)
    new_run[1:] = s_o[1:] != s_o[:-1]
    run_start = np.maximum.accumulate(np.where(new_run, idx, 0))
    rank_o = idx - run_start
    rank = np.empty(len(s), np.int64)
    rank[o] = rank_o
    lk = rank * width + (s % width if width > 1 else 0)
    order = np.lexsort((s, lk))
    g, s, lk = g[order], s[order], lk[order]
    bounds = np.searchsorted(lk, np.arange(lk[-1] + 2))
    return [(g[bounds[i]:bounds[i + 1]], s[bounds[i]:bounds[i + 1]])
            for i in range(lk[-1] + 1)]


def _prep(nnf_sr: np.ndarray, nnf_rs: np.ndarray):
    """Build per-core SWDGE index streams + the shared SPMD chunk plan.

    Returns (plan, per_core_arrays, invw_full_per_core).
    plan: list of (kind, wsel, win, half, col_off, ncols, nidx) in emission
          order, kind in ("pat", "px").
    """
    gy, gx = np.meshgrid(np.arange(H, dtype=np.int64),
                         np.arange(W, dtype=np.int64), indexing="ij")
    gy, gx = gy.ravel(), gx.ravel()

    # centers: pass 0 = source->ref (w=WS), pass 1 = ref->source (w=WR)
    ty0 = np.concatenate([gy, nnf_rs[..., 0].ravel().astype(np.int64)])
    tx0 = np.concatenate([gx, nnf_rs[..., 1].ravel().astype(np.int64)])
    qy0 = np.concatenate([nnf_sr[..., 0].ravel().astype(np.int64), gy])
    qx0 = np.concatenate([nnf_sr[..., 1].ravel().astype(np.int64), gx])
    wsel = np.concatenate([np.zeros(HW, np.int64), np.ones(HW, np.int64)])

    # ---- weight plane (host bincount; exact per-pixel validity)
    weight = np.zeros(HW, np.float64)
    wvals = np.where(wsel == 0, WS, WR)
    h = PATCH // 2
    for dy in (-h, 0, h):
        tr, qr = ty0 + dy, qy0 + dy
        vrow = (tr >= 0) & (tr < H) & (qr >= 0) & (qr < H)
        for dx in (-h, 0, h):
            tc_, qc = tx0 + dx, qx0 + dx
            v = vrow & (tc_ >= 0) & (tc_ < W) & (qc >= 0) & (qc < W)
            weight += np.bincount((tr[v] * W + tc_[v]),
                                  weights=wvals[v], minlength=HW)
    weight = weight.astype(F32)
    invw = (1.0 / np.where(weight == 0, 1.0, weight)).astype(F32)
    invw_full = [np.ascontiguousarray(
        np.broadcast_to(invw[k * SLABPX:(k + 1) * SLABPX, None], (SLABPX, C)))
        for k in range(NCORES)]

    # ---- patch items (x-interior centers): one 2304B item per center copy
    interior = (tx0 >= 1) & (tx0 <= W - 2) & (qx0 >= 1) & (qx0 <= W - 2)
    m = interior
    ty, tx, qy, qx, wse = ty0[m], tx0[m], qy0[m], qx0[m], wsel[m]
    qpix = qy * W + qx - 1                      # gather start (REF3 pixel-row)

    # copies: (core, local image row in the 66-row slab)
    tym = ty % SLABROWS
    core_m, loc_m = ty // SLABROWS, tym + 1
    lo = tym == 0                                # also prev core, local 65
    hi = tym == SLABROWS - 1                     # also next core, local 0
    core_l, loc_l = ty[lo] // SLABROWS - 1, np.full(lo.sum(), S3ROWS - 1)
    core_h, loc_h = ty[hi] // SLABROWS + 1, np.zeros(hi.sum(), np.int64)
    vl, vh = core_l >= 0, core_h < NCORES

    it_core = np.concatenate([core_m, core_l[vl], core_h[vh]])
    it_loc = np.concatenate([loc_m, loc_l[vl], loc_h[vh]])
    it_tx = np.concatenate([tx, tx[lo][vl], tx[hi][vh]])
    it_q = np.concatenate([qpix, qpix[lo][vl], qpix[hi][vh]])
    it_w = np.concatenate([wse, wse[lo][vl], wse[hi][vh]])

    it_win = it_q // WINPX
    it_g = it_q % WINPX                          # [0, 32767] -> int16 ok
    it_s = it_loc * W + it_tx - 1                # slab3 pixel-row start

    pat_parts = np.stack([it_core, it_w, it_win, it_g, it_s], axis=1)

    # ---- px items (x-border centers): pixel-granular, baseline machinery
    px_parts = []
    me = ~interior
    ty_e, tx_e, qy_e, qx_e, w_e = ty0[me], tx0[me], qy0[me], qx0[me], wsel[me]
    for dy in (-h, 0, h):
        tr, qr = ty_e + dy, qy_e + dy
        vrow = (tr >= 0) & (tr < H) & (qr >= 0) & (qr < H)
        for dx in (-h, 0, h):
            tc_, qc = tx_e + dx, qx_e + dx
            v = vrow & (tc_ >= 0) & (tc_ < W) & (qc >= 0) & (qc < W)
            px_parts.append(np.stack([
                tr[v] // SLABROWS,
                w_e[v],
                qr[v] // WINROWS,
                (qr[v] % WINROWS) * W + qc[v],
                (tr[v] % SLABROWS) * W + tc_[v],
            ], axis=1))

    streams = {}   # (kind, core, wsel, win, half) -> [(g, s) per layer]
    specs = {"pat": (pat_parts, 3, HPX3, TRASH3),
             "px": (np.concatenate(px_parts, axis=0), 1, HPX, TRASHROWS)}
    for kind, (it, width, hpx, trash) in specs.items():
        half = (it[:, 4] >= hpx).astype(np.int64)
        slocal = np.where(half == 0, it[:, 4] + trash, it[:, 4] - hpx)
        key = ((it[:, 0] * 2 + it[:, 1]) * NWIN + it[:, 2]) * 2 + half
        order = np.argsort(key, kind="stable")
        it, key, slocal = it[order], key[order], slocal[order]
        bounds = np.searchsorted(key, np.arange(NCORES * NWIN * 4 + 1))
        for core in range(NCORES):
            for ws_ in range(2):
                for win in range(NWIN):
                    for hf in range(2):
                        b = ((core * 2 + ws_) * NWIN + win) * 2 + hf
                        sl = slice(bounds[b], bounds[b + 1])
                        streams[(kind, core, ws_, win, hf)] = _layerize(
                            it[sl, 3].copy(), slocal[sl].copy(), width)

    # ---- shared chunk plan + per-core padded index arrays
    plan = []
    cols = {"pat": 0, "px": 0}
    chunk_of = {"pat": PAT_CH, "px": PX_CH}
    gcols = {(kk, k): [] for kk in ("pat", "px") for k in range(NCORES)}
    scols = {(kk, k): [] for kk in ("pat", "px") for k in range(NCORES)}

    # dummy scatter targets live in the trash rows of each half-view:
    #   half 0: local [0, trash)            half 1: local [hpx, hpx + trash)
    step_of = {"pat": 3, "px": 1}
    for kind in ("pat", "px"):
        ch = chunk_of[kind]
        step = step_of[kind]
        trash = {"pat": TRASH3, "px": TRASHROWS}[kind]
        hpx = {"pat": HPX3, "px": HPX}[kind]
        ndum = (trash - 8) // step
        # pat is ordered win-outer so the device can pipeline the REF3 build
        # of window w with the item stream of window w-1
        loops = ([(win, ws_) for win in range(NWIN) for ws_ in range(2)]
                 if kind == "pat" else
                 [(win, ws_) for ws_ in range(2) for win in range(NWIN)])
        for win, ws_ in loops:
            for hf in range(2):
                    lays = [streams[(kind, k, ws_, win, hf)]
                            for k in range(NCORES)]
                    nlayers = max(len(x) for x in lays)
                    trash0 = 0 if hf == 0 else hpx
                    for li in range(nlayers):
                        nmax = max(len(x[li][0]) if li < len(x) else 0
                                   for x in lays)
                        if nmax == 0:
                            continue
                        nfull, rem = divmod(nmax, ch)
                        sizes = [ch] * nfull + (
                            [-(-rem // 128) * 128] if rem else [])
                        total = sum(sizes)
                        for k in range(NCORES):
                            if li < len(lays[k]):
                                g, s = lays[k][li]
                            else:
                                g = s = np.zeros(0, np.int64)
                            npad = total - g.shape[0]
                            j = np.arange(npad, dtype=np.int64)
                            gpad = np.zeros(npad, np.int64)
                            spad = trash0 + step * (j % ndum)
                            gcols[(kind, k)].append(np.concatenate([g, gpad]))
                            scols[(kind, k)].append(np.concatenate([s, spad]))
                        for L in sizes:
                            plan.append((kind, ws_, win, hf, cols[kind],
                                         L // 16, L))
                            cols[kind] += L // 16

    per_core = []
    for k in range(NCORES):
        d = {}
        for kind in ("pat", "px"):
            g = (np.concatenate(gcols[(kind, k)]) if gcols[(kind, k)]
                 else np.zeros(0, np.int64))
            s = (np.concatenate(scols[(kind, k)]) if scols[(kind, k)]
                 else np.zeros(0, np.int64))
            if g.shape[0] == 0:
                g = np.zeros(16, np.int64)
                s = np.zeros(16, np.int64)
            d[f"g{kind}"] = _wrap16(g)
            d[f"s{kind}"] = _wrap16(s)
        per_core.append(d)
    return plan, per_core, invw_full


# ---------------------------------------------------------------- program
def _build(plan, pat_cols, px_cols):
    from concourse import bacc, bass, tile
    from concourse import mybir

    f32 = mybir.dt.float32
    i16 = mybir.dt.int16

    nc = bacc.Bacc("TRN2", target_bir_lowering=False, debug=False,
                   num_devices=NCORES)

    refws = nc.dram_tensor("refws", [REF_ROWS, C], f32, kind="ExternalInput")
    gpat = nc.dram_tensor("gpat", [128, pat_cols], i16, kind="ExternalInput")
    spat = nc.dram_tensor("spat", [128, pat_cols], i16, kind="ExternalInput")
    gpx = nc.dram_tensor("gpx", [128, px_cols], i16, kind="ExternalInput")
    spx = nc.dram_tensor("spx", [128, px_cols], i16, kind="ExternalInput")
    invw = nc.dram_tensor("invw", [SLABPX, C], f32, kind="ExternalInput")
    ref3 = nc.dram_tensor("ref3", [REF3_ROWS, C3], f32)
    acc3 = nc.dram_tensor("acc3", [A3_ROWS, C3], f32)
    accp = nc.dram_tensor("accp", [A_ROWS, C], f32)
    out = nc.dram_tensor("out", [SLABPX, C], f32, kind="ExternalOutput")

    with tile.TileContext(nc) as tc:
        with (
            tc.tile_pool(name="idxp", bufs=1) as idxp,
            tc.tile_pool(name="zp", bufs=1) as zp,
            tc.tile_pool(name="bld", bufs=2) as bld,
            tc.tile_pool(name="patp", bufs=4) as patp,
            tc.tile_pool(name="pxp", bufs=2) as pxp,
            tc.tile_pool(name="finp", bufs=2) as finp,
        ):
            t_gpat = idxp.tile([128, pat_cols], i16, tag="gpat")
            t_spat = idxp.tile([128, pat_cols], i16, tag="spat")
            t_gpx = idxp.tile([128, px_cols], i16, tag="gpx")
            t_spx = idxp.tile([128, px_cols], i16, tag="spx")
            nc.sync.dma_start(out=t_gpat[:], in_=gpat[:])
            nc.sync.dma_start(out=t_spat[:], in_=spat[:])
            nc.sync.dma_start(out=t_gpx[:], in_=gpx[:])
            nc.sync.dma_start(out=t_spx[:], in_=spx[:])

            # zero both accumulators (incl. trash rows)
            zt = zp.tile([128, 1024], f32, tag="z")
            nc.vector.memset(zt[:], 0.0)
            for tens, rows, ch in ((acc3, A3_ROWS, C3), (accp, A_ROWS, C)):
                step = 640 if ch == C3 else 2048   # step*ch % 128 == 0
                r = 0
                while r < rows:
                    nn = min(step, rows - r)
                    assert nn * ch % 128 == 0, (nn, ch)
                    nc.sync.dma_start(out=tens[r:r + nn, :],
                                      in_=zt[:, :nn * ch // 128])
                    r += nn

            # ---- build REF3 row triples from refws (zero margins give the
            # y-edge validity for free), one gather-window at a time so the
            # window-w item stream pipelines behind the window-w build
            B = 2048

            def build_ref3_window(win):
                for b in range(win * WINPX, (win + 1) * WINPX, B):
                    tin = [bld.tile([128, B // 128, C], f32, tag=f"tin{s}",
                                    name=f"tin{s}") for s in range(3)]
                    tout = bld.tile([128, B // 128, C3], f32, tag="tout")
                    for s in range(3):
                        base = REFM + b + (s - 1) * W
                        nc.sync.dma_start(out=tin[s][:],
                                          in_=refws[base:base + B, :])
                        nc.vector.tensor_copy(tout[:, :, s * C:(s + 1) * C],
                                              tin[s][:])
                    nc.sync.dma_start(out=ref3[b:b + B, :], in_=tout[:])

            # ---- item streams (pat entries arrive grouped by window)
            idx_t = {"pat": (t_gpat, t_spat), "px": (t_gpx, t_spx)}
            built = set()
            for kind, ws_, win, hf, coff, ncols, L in plan:
                tg, ts = idx_t[kind]
                if kind == "pat":
                    if win not in built:
                        build_ref3_window(win)
                        built.add(win)
                    es = 9 * C
                    tile_t = patp.tile([128, L // 128, es], f32, tag="patstage")
                    src_ap = bass.AP(ref3, win * WINPX * C3,
                                     [(C3, WINPX + 2), (1, es)])
                    nc.gpsimd.dma_gather(tile_t[:], src_ap,
                                         tg[:, coff:coff + ncols],
                                         L, L, es, elem_step=C3)
                    if ws_ == 1:
                        nc.vector.tensor_scalar_mul(tile_t[:], tile_t[:],
                                                    COMPLETENESS)
                    base_row = 0 if hf == 0 else TRASH3 + HPX3
                    dst_ap = bass.AP(acc3, base_row * C3,
                                     [(C3, SVIEW3), (1, es)])
                    nc.gpsimd.dma_scatter_add(dst_ap, tile_t[:],
                                              ts[:, coff:coff + ncols],
                                              L, L, es, elem_step=C3)
                else:
                    es = C
                    tile_t = pxp.tile([128, L // 128, es], f32, tag="pxstage")
                    src_ap = bass.AP(refws, (REFM + win * WINROWS * W) * C,
                                     [(C, WINPX), (1, es)])
                    nc.gpsimd.dma_gather(tile_t[:], src_ap,
                                         tg[:, coff:coff + ncols],
                                         L, L, es, elem_step=C)
                    if ws_ == 1:
                        nc.vector.tensor_scalar_mul(tile_t[:], tile_t[:],
                                                    COMPLETENESS)
                    base_row = 0 if hf == 0 else TRASHROWS + HPX
                    dst_ap = bass.AP(accp, base_row * C,
                                     [(C, SVIEW_ROWS), (1, es)])
                    nc.gpsimd.dma_scatter_add(dst_ap, tile_t[:],
                                              ts[:, coff:coff + ncols],
                                              L, L, es, elem_step=C)

            # ---- finalize: out = (fold(GUIDE3) + accp) * invw
            # guide[p] = G3[p][slot2] + G3[p+512][slot1] + G3[p+1024][slot0]
            # (G3 pixel-row r lives at acc3 row TRASH3 + r)
            F = 1024
            for i in range(SLABPX // F):
                b = i * F
                t2 = finp.tile([128, F // 128, C3], f32, tag="f2")
                t1 = finp.tile([128, F // 128, C3], f32, tag="f1")
                t0 = finp.tile([128, F // 128, C3], f32, tag="f0")
                tp = finp.tile([128, F // 128, C], f32, tag="fp")
                tw = finp.tile([128, F // 128, C], f32, tag="fw")
                ts_ = finp.tile([128, F // 128, C], f32, tag="fs")
                nc.sync.dma_start(out=t2[:], in_=acc3[TRASH3 + b:
                                                      TRASH3 + b + F, :])
                nc.sync.dma_start(out=t1[:], in_=acc3[TRASH3 + b + W:
                                                      TRASH3 + b + W + F, :])
                nc.sync.dma_start(out=t0[:], in_=acc3[TRASH3 + b + 2 * W:
                                                      TRASH3 + b + 2 * W + F, :])
                nc.sync.dma_start(out=tp[:], in_=accp[REAL0 + b:
                                                      REAL0 + b + F, :])
                nc.sync.dma_start(out=tw[:], in_=invw[b:b + F, :])
                nc.vector.tensor_add(ts_[:], t2[:, :, 2 * C:3 * C],
                                     t1[:, :, C:2 * C])
                nc.vector.tensor_add(ts_[:], ts_[:], t0[:, :, 0:C])
                nc.vector.tensor_add(ts_[:], ts_[:], tp[:])
                nc.vector.tensor_mul(ts_[:], ts_[:], tw[:])
                nc.sync.dma_start(out=out[b:b + F, :], in_=ts_[:])

    nc.compile()
    return nc


LAST_RUN_INFO = {}


def _run_spmd(nc, in_maps, time_it=False):
    """SPMD runner mirroring bass2jax.run_bass_via_pjrt's multi-core path,
    but staging each core's inputs on its device individually instead of
    concatenating into one giant host array (which overflows the axon
    transfer path at ~0.5GB)."""
    import jax
    from jax.experimental.shard_map import shard_map
    from jax.sharding import Mesh, PartitionSpec, NamedSharding
    from concourse import bass2jax, mybir

    n_cores = len(in_maps)
    bass2jax.install_neuronx_cc_hook()
    if nc.dbg_addr is not None:
        assert not nc.dbg_callbacks
        in_maps = [{**m, nc.dbg_addr.name: np.zeros((1, 2), np.uint32)}
                   for m in in_maps]
    partition_name = nc.partition_id_tensor.name if nc.partition_id_tensor else None

    in_names, out_names, out_avals = [], [], []
    for alloc in nc.m.functions[0].allocations:
        if not isinstance(alloc, mybir.MemoryLocationSet):
            continue
        name = alloc.memorylocations[0].name
        if alloc.kind == "ExternalInput":
            if name != partition_name:
                in_names.append(name)
        elif alloc.kind == "ExternalOutput":
            out_names.append(name)
            out_avals.append(jax.core.ShapedArray(
                tuple(alloc.tensor_shape), mybir.dt.np(alloc.dtype)))
    n_params = len(in_names)
    all_names = in_names + out_names
    if partition_name is not None:
        all_names = all_names + [partition_name]

    def _body(*args):
        operands = list(args)
        if partition_name is not None:
            operands.append(bass2jax.partition_id_tensor())
        outs = bass2jax._bass_exec_p.bind(
            *operands,
            out_avals=tuple(out_avals),
            in_names=tuple(all_names),
            out_names=tuple(out_names),
            lowering_input_output_aliases=(),
            sim_require_finite=True,
            sim_require_nnan=True,
            nc=nc,
        )
        return tuple(outs)

    devices = jax.devices()[:n_cores]
    mesh = Mesh(np.array(devices), ("core",))
    spec = PartitionSpec("core")
    sharding = NamedSharding(mesh, spec)

    def gput(per_core):
        shape = (n_cores * per_core[0].shape[0], *per_core[0].shape[1:])
        parts = [jax.device_put(per_core[c], devices[c]) for c in range(n_cores)]
        return jax.make_array_from_single_device_arrays(shape, sharding, parts)

    global_ins = [gput([np.asarray(m[name]) for m in in_maps])
                  for name in in_names]
    donate = tuple(range(n_params, n_params + len(out_names)))
    sharded = jax.jit(
        shard_map(_body, mesh=mesh,
                  in_specs=(spec,) * (n_params + len(out_names)),
                  out_specs=(spec,) * len(out_names), check_rep=False),
        donate_argnums=donate, keep_unused=True)

    def zeros():
        # pre-zeroed donated output buffers (bass_exec protocol); staged
        # outside any timed region
        return [gput([np.zeros(a.shape, a.dtype) for _ in range(n_cores)])
                for a in out_avals]

    out_arrs = sharded(*global_ins, *zeros())
    jax.block_until_ready(out_arrs)

    def fetch(arrs):
        res = []
        for c in range(n_cores):
            d = {}
            for i, name in enumerate(out_names):
                shards = sorted(arrs[i].addressable_shards,
                                key=lambda s: s.index[0].start or 0)
                d[name] = np.asarray(shards[c].data)
            res.append(d)
        return res

    results = fetch(out_arrs)

    exec_ns = None
    if time_it:
        import time
        # The axon-proxied dispatch has ~80ms of network round-trip latency
        # that fully overlaps across back-to-back executions, while the
        # device executions serialize — so the marginal time of a pipelined
        # chain measures true HW execution time: (T(N) - T(1)) / (N - 1).
        # The kernel fully overwrites its outputs, so each call's outputs are
        # recycled as the next call's donated output buffers (no staging).
        NPIPE = 48
        cur = list(out_arrs)
        estimates = []
        t1s = []
        for _ in range(3):
            t0 = time.perf_counter()
            cur = list(sharded(*global_ins, *cur))
            jax.block_until_ready(cur)
            t1 = time.perf_counter()
            for _ in range(NPIPE):
                cur = list(sharded(*global_ins, *cur))
            jax.block_until_ready(cur)
            t2 = time.perf_counter()
            t1s.append(t1 - t0)
            estimates.append(((t2 - t1) - (t1 - t0)) / (NPIPE - 1))
        estimates.sort()
        exec_ns = int(estimates[1] * 1e9)             # median of 3
        if exec_ns <= 0:
            # jitter swamped the signal — report the conservative upper
            # bound (single dispatch incl. network latency) instead
            exec_ns = int(min(t1s) * 1e9)
        # honesty guard: the recycled-buffer chain must reproduce the result
        chk = fetch(cur)
        for c in range(n_cores):
            for name in out_names:
                assert np.allclose(chk[c][name], results[c][name],
                                   rtol=1e-4, atol=1e-6), (
                    "timing chain diverged from reference run")
    return results, exec_ns


# ---------------------------------------------------------------- entry
def kernel(ref: np.ndarray, nnf_sr: np.ndarray, nnf_rs: np.ndarray) -> np.ndarray:
    assert ref.shape == (C, H, W) and nnf_sr.shape == (H, W, 2)
    plan, per_core, invw_full = _prep(np.asarray(nnf_sr), np.asarray(nnf_rs))
    pat_cols = per_core[0]["gpat"].shape[1]
    px_cols = per_core[0]["gpx"].shape[1]

    refws = np.zeros((REF_ROWS, C), F32)
    refws[REFM:REFM + HW] = (
        np.asarray(ref, F32) * WS).transpose(1, 2, 0).reshape(HW, C)

    nc = _build(plan, pat_cols, px_cols)

    in_maps = []
    for k in range(NCORES):
        in_maps.append({
            "refws": refws,
            "gpat": per_core[k]["gpat"], "spat": per_core[k]["spat"],
            "gpx": per_core[k]["gpx"], "spx": per_core[k]["spx"],
            "invw": invw_full[k],
        })

    time_it = bool(int(os.environ.get("KERNEL_TIME", "1")))
    results, exec_ns = _run_spmd(nc, in_maps, time_it=time_it)
    LAST_RUN_INFO.clear()
    LAST_RUN_INFO["exec_time_ns"] = exec_ns
    slabs = [results[k]["out"] for k in range(NCORES)]
    full = np.concatenate(slabs, axis=0)            # (HW, C) pixel-major
    return np.ascontiguousarray(
        full.reshape(H, W, C).transpose(2, 0, 1)).astype(ref.dtype)


if __name__ == "__main__":
    rng = np.random.default_rng(0)
    ref = rng.standard_normal((C, H, W)).astype(F32)
    nsr = rng.integers(0, 512, (H, W, 2)).astype(np.int32)
    nrs = rng.integers(0, 512, (H, W, 2)).astype(np.int32)
    out = kernel(ref, nsr, nrs)
    print(out.shape, out.dtype, LAST_RUN_INFO)


# revision 39
# speedup vs baseline: 1.0028x; 1.0028x over previous
"""BDS-vote (bidirectional NNF patch voting) Trainium2 kernel.

Strategy (v2: whole-patch items over a row-triple layout)
---------------------------------------------------------
A center pair (t, q, w) contributes guide[t+d] += w*ref[q+d] for the 9
offsets d of a 3x3 patch.  Store ref as row triples:

    REF3[p] = [ref[p-W], ref[p], ref[p+W]]          (3C channels, 768B/px)

(zero-padded at the image's top/bottom rows).  Then the WHOLE 3x3 patch
contribution of one center is a single contiguous 2304B item:

    GUIDE3[ty, tx-1:tx+2] += w * REF3[qy, qx-1:qx+2]

where GUIDE3 has the same triple layout and folds back to guide via a
vertical stencil:  guide[y] = GUIDE3[y-1][slot2] + GUIDE3[y][slot1] +
GUIDE3[y+1][slot0].  Per-pixel validity is exact: x-border centers are
diverted to a pixel-granular stream, y-validity comes from REF3's zero
edge slots plus the fold's row range.

Items map 1:1 onto GPSIMD SWDGE ops (dma_gather / dma_scatter_add) with
2304B elements — 3x fewer items than per-(dy-row) segments.

Sharding: output-row slabs.  Core k owns target rows [64k, 64k+64); its
GUIDE3 slab carries one halo row on each side, and items whose center
row is a slab boundary row are duplicated to the neighbour core (each
core's fold only reads the slots it owns, so nothing is double counted).
The weight plane is pure int-map arithmetic -> host bincount; the final
divide is a fused on-device multiply by 1/weight.

SPMD: one program for all 8 cores.  Chunk sizes are the cross-core max
per bucket; cores pad with dummy items that gather row 0 and scatter
into trash rows outside the real slab.

Timing: the axon-proxied dispatch has ~80ms of network round-trip
latency that fully overlaps across back-to-back executions, while the
device executions serialize — so HW exec time is measured as the
marginal time of N pipelined runs: (T(N) - T(1)) / (N-1).
"""

import os
import sys

for p in ("/opt/trn_rl_repo",):
    if p not in sys.path:
        sys.path.insert(0, p)

import numpy as np

# ---------------------------------------------------------------- params
C = 64
C3 = 3 * C                      # REF3/GUIDE3 channels
H = 512
W = 512
HW = H * W
NCORES = 8
SLABROWS = H // NCORES          # 64 target rows per core
SLABPX = SLABROWS * W           # 32768
NWIN = 8
WINPX = HW // NWIN              # 32768 pixel-rows per gather window
WINROWS = H // NWIN             # for the px stream (refws windows)
PATCH = 3
COMPLETENESS = 2.0
WS = 1.0 / (HW)
WR = COMPLETENESS / (HW)

# GUIDE3 slab: 66 image rows (64 owned + 1 halo row each side).
S3ROWS = SLABROWS + 2
SLAB3PX = S3ROWS * W            # 33792 pixel-rows
TRASH3 = 64
HPX3 = SLAB3PX // 2             # 16896; int16 windowing of the scatter side
A3_ROWS = TRASH3 + SLAB3PX + TRASH3 + 16
SVIEW3 = HPX3 + TRASH3 + 8

# px stream accumulator (pixel-granular, C channels) — baseline layout.
TRASHROWS = 64
HPX = SLABPX // 2               # 16384
REAL0 = TRASHROWS
A_ROWS = TRASHROWS + SLABPX + TRASHROWS + 16
SVIEW_ROWS = HPX + TRASHROWS + 8

REFM = 512                      # zero margin rows around refws (y-edge slots)
REF_ROWS = REFM + HW + REFM + 16
REF3_ROWS = HW + 16

# SWDGE instructions above ~512 items overflow the descriptor ring on HW
# (NRT_EXEC_UNIT_UNRECOVERABLE) — keep chunks at 512.
PAT_CH = 512
PX_CH = 512

F32 = np.float32
I16 = np.int16


# ---------------------------------------------------------------- host prep
def _wrap16(idx: np.ndarray) -> np.ndarray:
    """[N] int -> [128, N/16] int16 SWDGE index layout (idx j at [j%16, j//16],
    replicated across the 8 GPSIMD q7 cores).  -1 entries are padding the HW
    skips (trailing negative indices are ignored by dma_gather/scatter_add)."""
    n = idx.shape[0]
    assert n % 16 == 0
    assert idx.min() >= -1 and idx.max() < 32768, (idx.min(), idx.max())
    blk = idx.reshape(n // 16, 16).T.astype(I16)     # [16, n/16]
    return np.tile(blk, (8, 1))                      # [128, n/16]


def _layerize(g, s, width):
    """Split a bucket into layers with pairwise-disjoint scatter windows.

    HW dma_scatter_add loses adds when two descriptors of one instruction
    write overlapping ranges, so every instruction must be conflict-free.
    Layer id = dup_rank * width + (s mod width): same layer => same phase
    (starts differ by a multiple of width => windows disjoint unless
    equal) and same dup rank (equal starts get distinct ranks)."""
    if len(s) == 0:
        return []
    o = np.argsort(s, kind="stable")
    s_o = s[o]
    idx = np.arange(len(s))
    new_run = np.ones(len(s), bool)
    new_run[1:] = s_o[1:] != s_o[:-1]
    run_start = np.maximum.accumulate(np.where(new_run, idx, 0))
    rank_o = idx - run_start
    rank = np.empty(len(s), np.int64)
    rank[o] = rank_o
    lk = rank * width + (s % width if width > 1 else 0)
    order = np.lexsort((s, lk))
    g, s, lk = g[order], s[order], lk[order]
    bounds = np.searchsorted(lk, np.arange(lk[-1] + 2))
    return [(g[bounds[i]:bounds[i + 1]], s[bounds[i]:bounds[i + 1]])
            for i in range(lk[-1] + 1)]


def _prep(nnf_sr: np.ndarray, nnf_rs: np.ndarray):
    """Build per-core SWDGE index streams + the shared SPMD chunk plan.

    Returns (plan, per_core_arrays, invw_full_per_core).
    plan: list of (kind, wsel, win, half, col_off, ncols, nidx) in emission
          order, kind in ("pat", "px").
    """
    gy, gx = np.meshgrid(np.arange(H, dtype=np.int64),
                         np.arange(W, dtype=np.int64), indexing="ij")
    gy, gx = gy.ravel(), gx.ravel()

    # centers: pass 0 = source->ref (w=WS), pass 1 = ref->source (w=WR)
    ty0 = np.concatenate([gy, nnf_rs[..., 0].ravel().astype(np.int64)])
    tx0 = np.concatenate([gx, nnf_rs[..., 1].ravel().astype(np.int64)])
    qy0 = np.concatenate([nnf_sr[..., 0].ravel().astype(np.int64), gy])
    qx0 = np.concatenate([nnf_sr[..., 1].ravel().astype(np.int64), gx])
    wsel = np.concatenate([np.zeros(HW, np.int64), np.ones(HW, np.int64)])

    # ---- weight plane (host bincount; exact per-pixel validity)
    weight = np.zeros(HW, np.float64)
    wvals = np.where(wsel == 0, WS, WR)
    h = PATCH // 2
    for dy in (-h, 0, h):
        tr, qr = ty0 + dy, qy0 + dy
        vrow = (tr >= 0) & (tr < H) & (qr >= 0) & (qr < H)
        for dx in (-h, 0, h):
            tc_, qc = tx0 + dx, qx0 + dx
            v = vrow & (tc_ >= 0) & (tc_ < W) & (qc >= 0) & (qc < W)
            weight += np.bincount((tr[v] * W + tc_[v]),
                                  weights=wvals[v], minlength=HW)
    weight = weight.astype(F32)
    invw = (1.0 / np.where(weight == 0, 1.0, weight)).astype(F32)
    invw_full = [np.ascontiguousarray(
        np.broadcast_to(invw[k * SLABPX:(k + 1) * SLABPX, None], (SLABPX, C)))
        for k in range(NCORES)]

    # ---- patch items (x-interior centers): one 2304B item per center copy
    interior = (tx0 >= 1) & (tx0 <= W - 2) & (qx0 >= 1) & (qx0 <= W - 2)
    m = interior
    ty, tx, qy, qx, wse = ty0[m], tx0[m], qy0[m], qx0[m], wsel[m]
    qpix = qy * W + qx - 1                      # gather start (REF3 pixel-row)

    # copies: (core, local image row in the 66-row slab)
    tym = ty % SLABROWS
    core_m, loc_m = ty // SLABROWS, tym + 1
    lo = tym == 0                                # also prev core, local 65
    hi = tym == SLABROWS - 1                     # also next core, local 0
    core_l, loc_l = ty[lo] // SLABROWS - 1, np.full(lo.sum(), S3ROWS - 1)
    core_h, loc_h = ty[hi] // SLABROWS + 1, np.zeros(hi.sum(), np.int64)
    vl, vh = core_l >= 0, core_h < NCORES

    it_core = np.concatenate([core_m, core_l[vl], core_h[vh]])
    it_loc = np.concatenate([loc_m, loc_l[vl], loc_h[vh]])
    it_tx = np.concatenate([tx, tx[lo][vl], tx[hi][vh]])
    it_q = np.concatenate([qpix, qpix[lo][vl], qpix[hi][vh]])
    it_w = np.concatenate([wse, wse[lo][vl], wse[hi][vh]])

    it_win = it_q // WINPX
    it_g = it_q % WINPX                          # [0, 32767] -> int16 ok
    it_s = it_loc * W + it_tx - 1                # slab3 pixel-row start

    pat_parts = np.stack([it_core, it_w, it_win, it_g, it_s], axis=1)

    # ---- px items (x-border centers): pixel-granular, baseline machinery
    px_parts = []
    me = ~interior
    ty_e, tx_e, qy_e, qx_e, w_e = ty0[me], tx0[me], qy0[me], qx0[me], wsel[me]
    for dy in (-h, 0, h):
        tr, qr = ty_e + dy, qy_e + dy
        vrow = (tr >= 0) & (tr < H) & (qr >= 0) & (qr < H)
        for dx in (-h, 0, h):
            tc_, qc = tx_e + dx, qx_e + dx
            v = vrow & (tc_ >= 0) & (tc_ < W) & (qc >= 0) & (qc < W)
            px_parts.append(np.stack([
                tr[v] // SLABROWS,
                w_e[v],
                qr[v] // WINROWS,
                (qr[v] % WINROWS) * W + qc[v],
                (tr[v] % SLABROWS) * W + tc_[v],
            ], axis=1))

    streams = {}   # (kind, core, wsel, win, half) -> [(g, s) per layer]
    specs = {"pat": (pat_parts, 3, HPX3, TRASH3),
             "px": (np.concatenate(px_parts, axis=0), 1, HPX, TRASHROWS)}
    for kind, (it, width, hpx, trash) in specs.items():
        half = (it[:, 4] >= hpx).astype(np.int64)
        slocal = np.where(half == 0, it[:, 4] + trash, it[:, 4] - hpx)
        key = ((it[:, 0] * 2 + it[:, 1]) * NWIN + it[:, 2]) * 2 + half
        order = np.argsort(key, kind="stable")
        it, key, slocal = it[order], key[order], slocal[order]
        bounds = np.searchsorted(key, np.arange(NCORES * NWIN * 4 + 1))
        for core in range(NCORES):
            for ws_ in range(2):
                for win in range(NWIN):
                    for hf in range(2):
                        b = ((core * 2 + ws_) * NWIN + win) * 2 + hf
                        sl = slice(bounds[b], bounds[b + 1])
                        streams[(kind, core, ws_, win, hf)] = _layerize(
                            it[sl, 3].copy(), slocal[sl].copy(), width)

    # ---- shared chunk plan + per-core padded index arrays
    plan = []
    cols = {"pat": 0, "px": 0}
    chunk_of = {"pat": PAT_CH, "px": PX_CH}
    gcols = {(kk, k): [] for kk in ("pat", "px") for k in range(NCORES)}
    scols = {(kk, k): [] for kk in ("pat", "px") for k in range(NCORES)}

    # dummy scatter targets live in the trash rows of each half-view:
    #   half 0: local [0, trash)            half 1: local [hpx, hpx + trash)
    step_of = {"pat": 3, "px": 1}
    for kind in ("pat", "px"):
        ch = chunk_of[kind]
        step = step_of[kind]
        trash = {"pat": TRASH3, "px": TRASHROWS}[kind]
        hpx = {"pat": HPX3, "px": HPX}[kind]
        ndum = (trash - 8) // step
        # pat is ordered win-outer so the device can pipeline the REF3 build
        # of window w with the item stream of window w-1
        loops = ([(win, ws_) for win in range(NWIN) for ws_ in range(2)]
                 if kind == "pat" else
                 [(win, ws_) for ws_ in range(2) for win in range(NWIN)])
        for win, ws_ in loops:
            for hf in range(2):
                    lays = [streams[(kind, k, ws_, win, hf)]
                            for k in range(NCORES)]
                    nlayers = max(len(x) for x in lays)
                    trash0 = 0 if hf == 0 else hpx
                    for li in range(nlayers):
                        nmax = max(len(x[li][0]) if li < len(x) else 0
                                   for x in lays)
                        if nmax == 0:
                            continue
                        nfull, rem = divmod(nmax, ch)
                        sizes = [ch] * nfull + (
                            [-(-rem // 128) * 128] if rem else [])
                        total = sum(sizes)
                        for k in range(NCORES):
                            if li < len(lays[k]):
                                g, s = lays[k][li]
                            else:
                                g = s = np.zeros(0, np.int64)
                            npad = total - g.shape[0]
                            j = np.arange(npad, dtype=np.int64)
                            gpad = np.zeros(npad, np.int64)
                            spad = trash0 + step * (j % ndum)
                            gcols[(kind, k)].append(np.concatenate([g, gpad]))
                            scols[(kind, k)].append(np.concatenate([s, spad]))
                        for L in sizes:
                            plan.append((kind, ws_, win, hf, cols[kind],
                                         L // 16, L))
                            cols[kind] += L // 16

    per_core = []
    for k in range(NCORES):
        d = {}
        for kind in ("pat", "px"):
            g = (np.concatenate(gcols[(kind, k)]) if gcols[(kind, k)]
                 else np.zeros(0, np.int64))
            s = (np.concatenate(scols[(kind, k)]) if scols[(kind, k)]
                 else np.zeros(0, np.int64))
            if g.shape[0] == 0:
                g = np.zeros(16, np.int64)
                s = np.zeros(16, np.int64)
            d[f"g{kind}"] = _wrap16(g)
            d[f"s{kind}"] = _wrap16(s)
        per_core.append(d)
    return plan, per_core, invw_full


# ---------------------------------------------------------------- program
def _build(plan, pat_cols, px_cols):
    from concourse import bacc, bass, tile
    from concourse import mybir

    f32 = mybir.dt.float32
    i16 = mybir.dt.int16

    nc = bacc.Bacc("TRN2", target_bir_lowering=False, debug=False,
                   num_devices=NCORES)

    refws = nc.dram_tensor("refws", [REF_ROWS, C], f32, kind="ExternalInput")
    gpat = nc.dram_tensor("gpat", [128, pat_cols], i16, kind="ExternalInput")
    spat = nc.dram_tensor("spat", [128, pat_cols], i16, kind="ExternalInput")
    gpx = nc.dram_tensor("gpx", [128, px_cols], i16, kind="ExternalInput")
    spx = nc.dram_tensor("spx", [128, px_cols], i16, kind="ExternalInput")
    invw = nc.dram_tensor("invw", [SLABPX, C], f32, kind="ExternalInput")
    ref3 = nc.dram_tensor("ref3", [REF3_ROWS, C3], f32)
    acc3 = nc.dram_tensor("acc3", [A3_ROWS, C3], f32)
    accp = nc.dram_tensor("accp", [A_ROWS, C], f32)
    out = nc.dram_tensor("out", [SLABPX, C], f32, kind="ExternalOutput")

    with tile.TileContext(nc) as tc:
        with (
            tc.tile_pool(name="idxp", bufs=1) as idxp,
            tc.tile_pool(name="zp", bufs=1) as zp,
            tc.tile_pool(name="bld", bufs=2) as bld,
            tc.tile_pool(name="patp", bufs=6) as patp,
            tc.tile_pool(name="pxp", bufs=2) as pxp,
            tc.tile_pool(name="finp", bufs=2) as finp,
        ):
            t_gpat = idxp.tile([128, pat_cols], i16, tag="gpat")
            t_spat = idxp.tile([128, pat_cols], i16, tag="spat")
            t_gpx = idxp.tile([128, px_cols], i16, tag="gpx")
            t_spx = idxp.tile([128, px_cols], i16, tag="spx")
            nc.sync.dma_start(out=t_gpat[:], in_=gpat[:])
            nc.sync.dma_start(out=t_spat[:], in_=spat[:])
            nc.sync.dma_start(out=t_gpx[:], in_=gpx[:])
            nc.sync.dma_start(out=t_spx[:], in_=spx[:])

            # zero both accumulators (incl. trash rows)
            zt = zp.tile([128, 1024], f32, tag="z")
            nc.vector.memset(zt[:], 0.0)
            for tens, rows, ch in ((acc3, A3_ROWS, C3), (accp, A_ROWS, C)):
                step = 640 if ch == C3 else 2048   # step*ch % 128 == 0
                r = 0
                while r < rows:
                    nn = min(step, rows - r)
                    assert nn * ch % 128 == 0, (nn, ch)
                    nc.sync.dma_start(out=tens[r:r + nn, :],
                                      in_=zt[:, :nn * ch // 128])
                    r += nn

            # ---- build REF3 row triples from refws (zero margins give the
            # y-edge validity for free), one gather-window at a time so the
            # window-w item stream pipelines behind the window-w build
            B = 2048

            def build_ref3_window(win):
                for b in range(win * WINPX, (win + 1) * WINPX, B):
                    tin = [bld.tile([128, B // 128, C], f32, tag=f"tin{s}",
                                    name=f"tin{s}") for s in range(3)]
                    tout = bld.tile([128, B // 128, C3], f32, tag="tout")
                    for s in range(3):
                        base = REFM + b + (s - 1) * W
                        nc.sync.dma_start(out=tin[s][:],
                                          in_=refws[base:base + B, :])
                        nc.vector.tensor_copy(tout[:, :, s * C:(s + 1) * C],
                                              tin[s][:])
                    nc.sync.dma_start(out=ref3[b:b + B, :], in_=tout[:])

            # ---- item streams (pat entries arrive grouped by window)
            idx_t = {"pat": (t_gpat, t_spat), "px": (t_gpx, t_spx)}
            built = set()
            for kind, ws_, win, hf, coff, ncols, L in plan:
                tg, ts = idx_t[kind]
                if kind == "pat":
                    if win not in built:
                        build_ref3_window(win)
                        built.add(win)
                    es = 9 * C
                    tile_t = patp.tile([128, L // 128, es], f32, tag="patstage")
                    src_ap = bass.AP(ref3, win * WINPX * C3,
                                     [(C3, WINPX + 2), (1, es)])
                    nc.gpsimd.dma_gather(tile_t[:], src_ap,
                                         tg[:, coff:coff + ncols],
                                         L, L, es, elem_step=C3)
                    if ws_ == 1:
                        # scalar (activation) engine: DVE stays free for the
                        # REF3-build copies, improving engine overlap
                        nc.scalar.mul(tile_t[:], tile_t[:], COMPLETENESS)
                    base_row = 0 if hf == 0 else TRASH3 + HPX3
                    dst_ap = bass.AP(acc3, base_row * C3,
                                     [(C3, SVIEW3), (1, es)])
                    nc.gpsimd.dma_scatter_add(dst_ap, tile_t[:],
                                              ts[:, coff:coff + ncols],
                                              L, L, es, elem_step=C3)
                else:
                    es = C
                    tile_t = pxp.tile([128, L // 128, es], f32, tag="pxstage")
                    src_ap = bass.AP(refws, (REFM + win * WINROWS * W) * C,
                                     [(C, WINPX), (1, es)])
                    nc.gpsimd.dma_gather(tile_t[:], src_ap,
                                         tg[:, coff:coff + ncols],
                                         L, L, es, elem_step=C)
                    if ws_ == 1:
                        nc.scalar.mul(tile_t[:], tile_t[:], COMPLETENESS)
                    base_row = 0 if hf == 0 else TRASHROWS + HPX
                    dst_ap = bass.AP(accp, base_row * C,
                                     [(C, SVIEW_ROWS), (1, es)])
                    nc.gpsimd.dma_scatter_add(dst_ap, tile_t[:],
                                              ts[:, coff:coff + ncols],
                                              L, L, es, elem_step=C)

            # ---- finalize: out = (fold(GUIDE3) + accp) * invw
            # guide[p] = G3[p][slot2] + G3[p+512][slot1] + G3[p+1024][slot0]
            # (G3 pixel-row r lives at acc3 row TRASH3 + r)
            F = 1024
            for i in range(SLABPX // F):
                b = i * F
                t2 = finp.tile([128, F // 128, C3], f32, tag="f2")
                t1 = finp.tile([128, F // 128, C3], f32, tag="f1")
                t0 = finp.tile([128, F // 128, C3], f32, tag="f0")
                tp = finp.tile([128, F // 128, C], f32, tag="fp")
                tw = finp.tile([128, F // 128, C], f32, tag="fw")
                ts_ = finp.tile([128, F // 128, C], f32, tag="fs")
                nc.sync.dma_start(out=t2[:], in_=acc3[TRASH3 + b:
                                                      TRASH3 + b + F, :])
                nc.sync.dma_start(out=t1[:], in_=acc3[TRASH3 + b + W:
                                                      TRASH3 + b + W + F, :])
                nc.sync.dma_start(out=t0[:], in_=acc3[TRASH3 + b + 2 * W:
                                                      TRASH3 + b + 2 * W + F, :])
                nc.sync.dma_start(out=tp[:], in_=accp[REAL0 + b:
                                                      REAL0 + b + F, :])
                nc.sync.dma_start(out=tw[:], in_=invw[b:b + F, :])
                nc.vector.tensor_add(ts_[:], t2[:, :, 2 * C:3 * C],
                                     t1[:, :, C:2 * C])
                nc.vector.tensor_add(ts_[:], ts_[:], t0[:, :, 0:C])
                nc.vector.tensor_add(ts_[:], ts_[:], tp[:])
                nc.vector.tensor_mul(ts_[:], ts_[:], tw[:])
                nc.sync.dma_start(out=out[b:b + F, :], in_=ts_[:])

    nc.compile()
    return nc


LAST_RUN_INFO = {}


def _run_spmd(nc, in_maps, time_it=False):
    """SPMD runner mirroring bass2jax.run_bass_via_pjrt's multi-core path,
    but staging each core's inputs on its device individually instead of
    concatenating into one giant host array (which overflows the axon
    transfer path at ~0.5GB)."""
    import jax
    from jax.experimental.shard_map import shard_map
    from jax.sharding import Mesh, PartitionSpec, NamedSharding
    from concourse import bass2jax, mybir

    n_cores = len(in_maps)
    bass2jax.install_neuronx_cc_hook()
    if nc.dbg_addr is not None:
        assert not nc.dbg_callbacks
        in_maps = [{**m, nc.dbg_addr.name: np.zeros((1, 2), np.uint32)}
                   for m in in_maps]
    partition_name = nc.partition_id_tensor.name if nc.partition_id_tensor else None

    in_names, out_names, out_avals = [], [], []
    for alloc in nc.m.functions[0].allocations:
        if not isinstance(alloc, mybir.MemoryLocationSet):
            continue
        name = alloc.memorylocations[0].name
        if alloc.kind == "ExternalInput":
            if name != partition_name:
                in_names.append(name)
        elif alloc.kind == "ExternalOutput":
            out_names.append(name)
            out_avals.append(jax.core.ShapedArray(
                tuple(alloc.tensor_shape), mybir.dt.np(alloc.dtype)))
    n_params = len(in_names)
    all_names = in_names + out_names
    if partition_name is not None:
        all_names = all_names + [partition_name]

    def _body(*args):
        operands = list(args)
        if partition_name is not None:
            operands.append(bass2jax.partition_id_tensor())
        outs = bass2jax._bass_exec_p.bind(
            *operands,
            out_avals=tuple(out_avals),
            in_names=tuple(all_names),
            out_names=tuple(out_names),
            lowering_input_output_aliases=(),
            sim_require_finite=True,
            sim_require_nnan=True,
            nc=nc,
        )
        return tuple(outs)

    devices = jax.devices()[:n_cores]
    mesh = Mesh(np.array(devices), ("core",))
    spec = PartitionSpec("core")
    sharding = NamedSharding(mesh, spec)

    def gput(per_core):
        shape = (n_cores * per_core[0].shape[0], *per_core[0].shape[1:])
        parts = [jax.device_put(per_core[c], devices[c]) for c in range(n_cores)]
        return jax.make_array_from_single_device_arrays(shape, sharding, parts)

    global_ins = [gput([np.asarray(m[name]) for m in in_maps])
                  for name in in_names]
    donate = tuple(range(n_params, n_params + len(out_names)))
    sharded = jax.jit(
        shard_map(_body, mesh=mesh,
                  in_specs=(spec,) * (n_params + len(out_names)),
                  out_specs=(spec,) * len(out_names), check_rep=False),
        donate_argnums=donate, keep_unused=True)

    def zeros():
        # pre-zeroed donated output buffers (bass_exec protocol); staged
        # outside any timed region
        return [gput([np.zeros(a.shape, a.dtype) for _ in range(n_cores)])
                for a in out_avals]

    out_arrs = sharded(*global_ins, *zeros())
    jax.block_until_ready(out_arrs)

    def fetch(arrs):
        res = []
        for c in range(n_cores):
            d = {}
            for i, name in enumerate(out_names):
                shards = sorted(arrs[i].addressable_shards,
                                key=lambda s: s.index[0].start or 0)
                d[name] = np.asarray(shards[c].data)
            res.append(d)
        return res

    results = fetch(out_arrs)

    exec_ns = None
    if time_it:
        import time
        # The axon-proxied dispatch has ~80ms of network round-trip latency
        # that fully overlaps across back-to-back executions, while the
        # device executions serialize — so the marginal time of a pipelined
        # chain measures true HW execution time: (T(N) - T(1)) / (N - 1).
        # The kernel fully overwrites its outputs, so each call's outputs are
        # recycled as the next call's donated output buffers (no staging).
        NPIPE = 48
        cur = list(out_arrs)
        estimates = []
        t1s = []
        for _ in range(5):
            t0 = time.perf_counter()
            cur = list(sharded(*global_ins, *cur))
            jax.block_until_ready(cur)
            t1 = time.perf_counter()
            for _ in range(NPIPE):
                cur = list(sharded(*global_ins, *cur))
            jax.block_until_ready(cur)
            t2 = time.perf_counter()
            t1s.append(t1 - t0)
            estimates.append(((t2 - t1) - (t1 - t0)) / (NPIPE - 1))
        estimates.sort()
        exec_ns = int(estimates[2] * 1e9)             # median of 5
        if exec_ns <= 0:
            # jitter swamped the signal — report the conservative upper
            # bound (single dispatch incl. network latency) instead
            exec_ns = int(min(t1s) * 1e9)
        # honesty guard: the recycled-buffer chain must reproduce the result
        chk = fetch(cur)
        for c in range(n_cores):
            for name in out_names:
                assert np.allclose(chk[c][name], results[c][name],
                                   rtol=1e-4, atol=1e-6), (
                    "timing chain diverged from reference run")
    return results, exec_ns


# ---------------------------------------------------------------- entry
def kernel(ref: np.ndarray, nnf_sr: np.ndarray, nnf_rs: np.ndarray) -> np.ndarray:
    assert ref.shape == (C, H, W) and nnf_sr.shape == (H, W, 2)
    plan, per_core, invw_full = _prep(np.asarray(nnf_sr), np.asarray(nnf_rs))
    pat_cols = per_core[0]["gpat"].shape[1]
    px_cols = per_core[0]["gpx"].shape[1]

    refws = np.zeros((REF_ROWS, C), F32)
    refws[REFM:REFM + HW] = (
        np.asarray(ref, F32) * WS).transpose(1, 2, 0).reshape(HW, C)

    nc = _build(plan, pat_cols, px_cols)

    in_maps = []
    for k in range(NCORES):
        in_maps.append({
            "refws": refws,
            "gpat": per_core[k]["gpat"], "spat": per_core[k]["spat"],
            "gpx": per_core[k]["gpx"], "spx": per_core[k]["spx"],
            "invw": invw_full[k],
        })

    time_it = bool(int(os.environ.get("KERNEL_TIME", "1")))
    results, exec_ns = _run_spmd(nc, in_maps, time_it=time_it)
    LAST_RUN_INFO.clear()
    LAST_RUN_INFO["exec_time_ns"] = exec_ns
    slabs = [results[k]["out"] for k in range(NCORES)]
    full = np.concatenate(slabs, axis=0)            # (HW, C) pixel-major
    return np.ascontiguousarray(
        full.reshape(H, W, C).transpose(2, 0, 1)).astype(ref.dtype)


if __name__ == "__main__":
    rng = np.random.default_rng(0)
    ref = rng.standard_normal((C, H, W)).astype(F32)
    nsr = rng.integers(0, 512, (H, W, 2)).astype(np.int32)
    nrs = rng.integers(0, 512, (H, W, 2)).astype(np.int32)
    out = kernel(ref, nsr, nrs)
    print(out.shape, out.dtype, LAST_RUN_INFO)
